# revision 15
# baseline (speedup 1.0000x reference)
"""BlockwiseDense Trainium2 kernel (8 NeuronCores, sharded over out_blocks).

Math (per reference):
    w = rram_quantize(relu(cores))          # snap to 256 log-spaced levels
    y[b,i,j,k] = sum_l w[i,j,k,l] * x[b,j,l]

The quantizer index s(w) = MULT*ln((A-w)/B) + C0 is approximated by the
quadratic s ~= A2M*w^2 + P1*w + P0 (~0.06% level flips).  Per j-block:
    gg = ts(w, A2M, P1)       DVE fp16 (2x rate)
    hh = tt(w, gg)            DVE fp16
    n  = sat_u8(hh + P0)      GpSimd (its one fast op class: fp-in ts
                              with plain ALU ops; MAX/tt are microcoded
                              ~2-14x slower there, and all-fp16 in/out
                              halves its rate, hence fp16->u8 ADD)
    e  = Exp(ln_r * n)        ACT fp16 out
relu is implicit: negative w gives s<0 which the saturating u8 cast
clamps to n=0, the g_min level, exactly matching relu+quantize.

fp16 matmuls (512-wide moving operand, half PE rate) accumulate y in
fp32 PSUM over the two 128-row halves of l; a ones(-1.0) matmul
accumulates s2x = -sum_l x per j into a PSUM column.  Evicts apply
y = A*s2 - B*(x@e): nine j's on DVE (tensor_scalar with per-partition
sa = -A/B*s2), seven on ACT (Identity with bias AP = A*s2) - Identity
and Exp share one ACT table set so no table reloads occur.  -A/B is not
fp16-representable so the scale always happens in fp32 on the tiny
[128, nj] sa tiles, never via the matmul ones value.

DMA: one hardware queue's descriptor feed sustains only ~140 GB/s, so
weight granules round-robin over the sync, tensor, and gpsimd rings
(x's two chunks interleave on gpsimd; y stores alternate sync/tensor).
Core c takes out_blocks {2c, 2c+1}.
"""

import numpy as np

import concourse.bacc as bacc
import concourse.mybir as mybir
from concourse.tile import TileContext
from concourse.bass_utils import run_bass_kernel_spmd

BATCH = 128
IN_BLOCKS = 16
OUT_BLOCKS = 16
NB = 256
N_CORES = 8
I_PER_CORE = OUT_BLOCKS // N_CORES  # 2
IK = I_PER_CORE * NB  # 512

TAU, G_INF, G_MIN, L = 0.75, 2.0, 0.001, 256
B_SCALE = (G_INF - G_MIN) / (1.0 - float(np.exp(-TAU)))
A_OFF = G_MIN + B_SCALE
MULT = -(L - 1) / TAU
LN_R = -TAU / (L - 1)

# quadratic fit s(w) ~= A2M*w^2 + P1*w + P0 (baseline constants)
C0 = 0.5 - float(np.log((1 + np.exp(LN_R)) / 2) / LN_R)
_c1 = -C0 / 340.0
_g2 = -340.0 - 170.0 * _c1
C1W = -(G_MIN + B_SCALE * _c1)
A2M = 170.0 / (B_SCALE * B_SCALE)
A2B = -(170.0 * G_MIN / B_SCALE + _g2) / B_SCALE
P1 = A2B + C1W * A2M
P0 = C1W * A2B

F32 = mybir.dt.float32
F16 = mybir.dt.float16
U8 = mybir.dt.uint8

# granules: (j-list, evict engine "v"=DVE / "a"=ACT Identity)
GSPEC = [
    ([0], "v"),
    ([1], "v"),
    ([2, 3], "a"),
    ([4, 5], "v"),
    ([6, 7], "a"),
    ([8, 9], "v"),
    ([10, 11], "a"),
    ([12, 13], "v"),
    ([14], "v"),
    ([15], "a"),
]
# weight-DMA issuing ring per granule ("s" sync / "a" scalar / "g" gpsimd);
# scalar's issues land in its dead window before the first Exp
WRING = {0: "s", 1: "a", 2: "g", 3: "s", 4: "a", 5: "s", 6: "a", 7: "s", 8: "s", 9: "g"}

_CACHE = {}


class _ForceExpIdentityTable:
    """Resolve Exp and Identity to the single table set containing both,
    so the ACT never reloads tables mid-kernel."""

    def __enter__(self):
        self._orig = bacc.get_activation_tables
        Exp = mybir.ActivationFunctionType.Exp
        Idn = mybir.ActivationFunctionType.Identity

        def patched(arch):
            tabs = self._orig(arch)
            out = {}
            for name, fns in tabs.items():
                if name != "exp_and_others" and (Exp in fns or Idn in fns):
                    fns = fns - {Exp, Idn}
                out[name] = fns
            return out

        bacc.get_activation_tables = patched
        return self

    def __exit__(self, *exc):
        bacc.get_activation_tables = self._orig


def _build():
    nc = bacc.Bacc(trn_type="TRN2")
    P = 128
    NG = len(GSPEC)

    xt_d = nc.dram_tensor("xt", [P, IN_BLOCKS, 2, BATCH], F16, kind="ExternalInput")
    wt_d = nc.dram_tensor("wt", [P, IN_BLOCKS, 2, IK], F16, kind="ExternalInput")
    y_d = nc.dram_tensor("y", [BATCH, IN_BLOCKS, IK], F16, kind="ExternalOutput")

    flat = "p a b k -> p (a b k)"
    MUL = mybir.AluOpType.mult
    ADD = mybir.AluOpType.add

    RING = {"s": nc.sync, "a": nc.scalar, "g": nc.gpsimd}

    with TileContext(nc) as tc:
        with (
            tc.tile_pool(name="singles", bufs=1) as singles,
            tc.tile_pool(name="wraw", bufs=NG) as wpool,
            tc.tile_pool(name="tv16", bufs=6) as vpool,
            tc.tile_pool(name="nidx", bufs=4) as npool,
            tc.tile_pool(name="texp", bufs=4) as epool,
            tc.tile_pool(name="sab", bufs=4) as spool,
            tc.tile_pool(name="yout", bufs=3) as ypool,
            tc.tile_pool(name="yps", bufs=6, space="PSUM") as yps,
            tc.tile_pool(name="sps", bufs=1, space="PSUM") as sps,
        ):
            wt_t = [None] * NG
            n_t = [None] * NG
            e_t = [None] * NG
            sab_t = [None] * NG
            y_t = [None] * NG
            p_t = [None] * IN_BLOCKS

            def dma_w(g):
                js, _ = GSPEC[g]
                nj = len(js)
                wt_t[g] = wpool.tile([P, nj, 2, IK], F16, name="wraw", tag="wraw")
                RING[WRING[g]].dma_start(
                    out=wt_t[g][:], in_=wt_d[:, js[0] : js[0] + nj]
                )

            def chain(g):
                js, _ = GSPEC[g]
                fd = len(js) * 2 * IK
                w = wt_t[g][:].rearrange(flat)
                gg = vpool.tile([P, fd], F16, name="gg16", tag="tv16")
                nc.vector.tensor_scalar(gg[:], w, A2M, P1, MUL, ADD)
                hh = vpool.tile([P, fd], F16, name="hh16", tag="tv16")
                nc.vector.tensor_tensor(hh[:], w, gg[:], MUL)
                n_t[g] = npool.tile([P, fd], U8, name="nidx", tag="nidx")
                nc.gpsimd.tensor_scalar(n_t[g][:], hh[:], P0, None, ADD)

            def exp_stage(g):
                js, _ = GSPEC[g]
                nj = len(js)
                e_t[g] = epool.tile([P, nj, 2, IK], F16, name="texp", tag="texp")
                nc.scalar.activation(
                    e_t[g][:].rearrange(flat),
                    n_t[g][:],
                    mybir.ActivationFunctionType.Exp,
                    bias=0.0,
                    scale=LN_R,
                )

            def mm_stage(g):
                js, _ = GSPEC[g]
                for jrel, j in enumerate(js):
                    p_t[j] = yps.tile([P, IK], F32, name="yp", tag="yp")
                    for h in range(2):
                        nc.tensor.matmul(
                            s2_ps[:, j : j + 1],
                            xt_sb[:, j, h, :],
                            negones_sb[:],
                            start=(h == 0),
                            stop=(h == 1),
                        )
                        nc.tensor.matmul(
                            p_t[j][:],
                            xt_sb[:, j, h, :],
                            e_t[g][:, jrel, h, :],
                            start=(h == 0),
                            stop=(h == 1),
                        )

            def evict_stage(g):
                js, eng = GSPEC[g]
                nj = len(js)
                y_t[g] = ypool.tile([P, nj, IK], F16, name="ysb", tag="ysb")
                # s2_ps holds s2x = -s2 (ones preset to -1.0, exact in f16);
                # -A/B is not f16-representable so scale here in f32
                sab_t[g] = spool.tile([P, nj], F32, name="sab", tag="sab")
                if eng == "v":
                    # sa = -A/B*s2, then y = (p + sa)*(-B)
                    nc.vector.tensor_scalar(
                        sab_t[g][:],
                        s2_ps[:, js[0] : js[0] + nj],
                        A_OFF / B_SCALE,
                        None,
                        MUL,
                    )
                    for jrel, j in enumerate(js):
                        nc.vector.tensor_scalar(
                            y_t[g][:, jrel, :],
                            p_t[j][:],
                            sab_t[g][:, jrel : jrel + 1],
                            -B_SCALE,
                            ADD,
                            MUL,
                        )
                else:
                    # sab = A*s2 (SBUF), then ACT: y = -B*p + sab
                    nc.vector.tensor_scalar(
                        sab_t[g][:],
                        s2_ps[:, js[0] : js[0] + nj],
                        -A_OFF,
                        None,
                        MUL,
                    )
                    for jrel, j in enumerate(js):
                        nc.scalar.activation(
                            y_t[g][:, jrel, :],
                            p_t[j][:],
                            mybir.ActivationFunctionType.Identity,
                            bias=sab_t[g][:, jrel : jrel + 1],
                            scale=-B_SCALE,
                        )

            def store_stage(g):
                js, _ = GSPEC[g]
                nc.sync.dma_start(
                    out=y_d[:, js[0] : js[0] + len(js)], in_=y_t[g][:]
                )

            # --- prologue ---
            # tiny Exp forces the ACT table load before real work
            warm = singles.tile([P, 1], F32)
            nc.scalar.activation(
                warm[:], warm[:], mybir.ActivationFunctionType.Exp,
                bias=0.0, scale=0.0,
            )
            negones_sb = singles.tile([P, 1], F16)
            nc.vector.memset(negones_sb[:], -1.0)
            s2_ps = sps.tile([P, IN_BLOCKS], F32)
            warm_l = singles.tile([P, 16], F16)
            nc.vector.memset(warm_l[:], 0.5)
            warm_r = singles.tile([P, IK], F16)
            nc.vector.memset(warm_r[:], 0.5)
            wm_ps = sps.tile([16, IK], F32)

            # input DMAs upfront, spread over three rings; x's two chunks
            # interleave with weights on the gpsimd ring
            xt_sb = singles.tile([P, IN_BLOCKS, 2, BATCH], F16)
            nc.gpsimd.dma_start(out=xt_sb[:, 0:8], in_=xt_d[:, 0:8])
            dma_w(0)   # sync
            dma_w(1)   # scalar
            dma_w(2)   # gpsimd
            dma_w(3)   # sync
            dma_w(4)   # scalar
            nc.gpsimd.dma_start(out=xt_sb[:, 8:16], in_=xt_d[:, 8:16])
            dma_w(5)   # sync
            dma_w(6)   # scalar
            dma_w(7)   # sync
            dma_w(8)   # sync
            dma_w(9)   # gpsimd

            # PE warm-up raises the HAM clock gate to 2.4 GHz
            for _ in range(8):
                nc.tensor.matmul(
                    wm_ps[:], warm_l[:], warm_r[:], start=True, stop=True
                )

            # --- pipelined main loop ---
            chain(0)
            chain(1)
            for g in range(NG):
                if g + 2 < NG:
                    chain(g + 2)
                exp_stage(g)
                mm_stage(g)
                if g >= 1:
                    evict_stage(g - 1)
                    store_stage(g - 1)
            evict_stage(NG - 1)
            store_stage(NG - 1)

    with _ForceExpIdentityTable():
        nc.compile()
    return nc


def _get_nc():
    if "nc" not in _CACHE:
        _CACHE["nc"] = _build()
    return _CACHE["nc"]


def kernel(x: np.ndarray, cores: np.ndarray, _trace=False, _trace_kwargs=None):
    x = np.asarray(x, dtype=np.float32)
    cores = np.asarray(cores, dtype=np.float32)

    xt = np.ascontiguousarray(
        x.T.reshape(IN_BLOCKS, 2, 128, BATCH)
        .transpose(2, 0, 1, 3)
        .astype(np.float16)
    )
    wt_full = (
        cores.reshape(OUT_BLOCKS, IN_BLOCKS, NB, 2, 128)  # i, j, k, h, p
        .transpose(4, 1, 3, 0, 2)  # p, j, h, i, k
        .astype(np.float16)
    )

    in_maps = []
    for c in range(N_CORES):
        wc = np.ascontiguousarray(
            wt_full[:, :, :, c * I_PER_CORE : (c + 1) * I_PER_CORE, :]
        ).reshape(128, IN_BLOCKS, 2, IK)
        in_maps.append({"xt": xt, "wt": wc})

    nc = _get_nc()
    kw = {}
    if _trace:
        kw = dict(trace=True, **(_trace_kwargs or {}))
    out = run_bass_kernel_spmd(nc, in_maps, core_ids=list(range(N_CORES)), **kw)
    if _trace:
        _CACHE["last_result"] = out
    y = np.concatenate(
        [
            r["y"]  # (b, j, (i,k))
            .astype(np.float32)
            .reshape(BATCH, IN_BLOCKS, I_PER_CORE, NB)
            .transpose(0, 2, 1, 3)
            for r in out.results
        ],
        axis=1,
    )
    return y


# revision 16
# speedup vs baseline: 1.0075x; 1.0075x over previous
"""BlockwiseDense Trainium2 kernel (8 NeuronCores, sharded over out_blocks).

Math (per reference):
    w = rram_quantize(relu(cores))          # snap to 256 log-spaced levels
    y[b,i,j,k] = sum_l w[i,j,k,l] * x[b,j,l]

The quantizer index s(w) = MULT*ln((A-w)/B) + C0 is approximated by the
quadratic s ~= A2M*w^2 + P1*w + P0 (~0.06% level flips).  Per j-block:
    gg = ts(w, A2M, P1)       DVE fp16 (2x rate)
    hh = tt(w, gg)            DVE fp16
    n  = sat_u8(hh + P0)      GpSimd (its one fast op class: plain-ALU
                              ts with a non-fp16 side; MAX and tt are
                              microcoded there, 2-14x slower)
    e  = Exp(ln_r * n)        ACT fp16 out
relu is implicit: negative w gives s<0 which the saturating u8 cast
clamps to n=0, the g_min level, exactly matching relu+quantize.

fp16 matmuls (512-wide moving operand, half PE rate) accumulate y in
fp32 PSUM over the two 128-row halves of l; a ones(-1.0) matmul
accumulates s2x = -sum_l x per j into a PSUM column.  Evicts apply
y = A*s2 - B*(x@e): nine j's on DVE (tensor_scalar with per-partition
sa = -A/B*s2), seven on ACT (Identity with bias AP = A*s2); Identity
and Exp share one ACT table set so no reloads occur.  -A/B is not
fp16-representable, so the scale happens in fp32 on tiny [128,nj] sa
tiles, never via the matmul ones value.

DMA: each queue adds ~1.2us fixed overhead per DMA instruction and a
single queue feeds ~140 GB/s, so weights ship as SIX large transfers
(0.25-1 MB) split across the sync and gpsimd rings, issued upfront and
sliced by the compute granules; x's two chunks interleave on gpsimd;
y lives in one contiguous SBUF tile and ships as five batched stores
on the sync ring.  Core c takes out_blocks {2c, 2c+1}.
"""

import numpy as np

import concourse.bacc as bacc
import concourse.mybir as mybir
from concourse.tile import TileContext
from concourse.bass_utils import run_bass_kernel_spmd

BATCH = 128
IN_BLOCKS = 16
OUT_BLOCKS = 16
NB = 256
N_CORES = 8
I_PER_CORE = OUT_BLOCKS // N_CORES  # 2
IK = I_PER_CORE * NB  # 512

TAU, G_INF, G_MIN, L = 0.75, 2.0, 0.001, 256
B_SCALE = (G_INF - G_MIN) / (1.0 - float(np.exp(-TAU)))
A_OFF = G_MIN + B_SCALE
MULT = -(L - 1) / TAU
LN_R = -TAU / (L - 1)

# quadratic fit s(w) ~= A2M*w^2 + P1*w + P0 (baseline constants)
C0 = 0.5 - float(np.log((1 + np.exp(LN_R)) / 2) / LN_R)
_c1 = -C0 / 340.0
_g2 = -340.0 - 170.0 * _c1
C1W = -(G_MIN + B_SCALE * _c1)
A2M = 170.0 / (B_SCALE * B_SCALE)
A2B = -(170.0 * G_MIN / B_SCALE + _g2) / B_SCALE
P1 = A2B + C1W * A2M
P0 = C1W * A2B

F32 = mybir.dt.float32
F16 = mybir.dt.float16
U8 = mybir.dt.uint8

# processing granules: (j-list, evict engine "v"=DVE / "a"=ACT Identity)
GSPEC = [
    ([0], "v"),
    ([1], "v"),
    ([2, 3], "a"),
    ([4, 5], "v"),
    ([6, 7], "a"),
    ([8, 9], "v"),
    ([10, 11], "a"),
    ([12, 13], "v"),
    ([14], "v"),
    ([15], "a"),
]
# DMA granules: (j0, nj, ring) in issue order; rings "s"=sync, "g"=gpsimd.
# "x0"/"x1" are the two x chunks on the gpsimd ring.
DMA_PLAN = ["x0", (0, 1, "s"), (1, 1, "s"), (2, 2, "s"), (4, 4, "g"),
            "x1", (8, 4, "s"), (12, 2, "g"), (14, 2, "s")]
# batched y stores: (j0, nj) after all their granules evicted
STORES = [(0, 2), (2, 4), (6, 4), (10, 4), (14, 2)]

_CACHE = {}


class _ForceExpIdentityTable:
    """Resolve Exp and Identity to the single table set containing both,
    so the ACT never reloads tables mid-kernel."""

    def __enter__(self):
        self._orig = bacc.get_activation_tables
        Exp = mybir.ActivationFunctionType.Exp
        Idn = mybir.ActivationFunctionType.Identity

        def patched(arch):
            tabs = self._orig(arch)
            out = {}
            for name, fns in tabs.items():
                if name != "exp_and_others" and (Exp in fns or Idn in fns):
                    fns = fns - {Exp, Idn}
                out[name] = fns
            return out

        bacc.get_activation_tables = patched
        return self

    def __exit__(self, *exc):
        bacc.get_activation_tables = self._orig


def _build():
    nc = bacc.Bacc(trn_type="TRN2")
    P = 128
    NG = len(GSPEC)

    xt_d = nc.dram_tensor("xt", [P, IN_BLOCKS, 2, BATCH], F16, kind="ExternalInput")
    wt_d = nc.dram_tensor("wt", [P, IN_BLOCKS, 2, IK], F16, kind="ExternalInput")
    y_d = nc.dram_tensor("y", [BATCH, IN_BLOCKS, IK], F16, kind="ExternalOutput")

    flat = "p a b k -> p (a b k)"
    MUL = mybir.AluOpType.mult
    ADD = mybir.AluOpType.add

    with TileContext(nc) as tc:
        with (
            tc.tile_pool(name="singles", bufs=1) as singles,
            tc.tile_pool(name="tv16", bufs=6) as vpool,
            tc.tile_pool(name="nidx", bufs=4) as npool,
            tc.tile_pool(name="texp", bufs=4) as epool,
            tc.tile_pool(name="sab", bufs=4) as spool,
            tc.tile_pool(name="yps", bufs=6, space="PSUM") as yps,
            tc.tile_pool(name="sps", bufs=1, space="PSUM") as sps,
        ):
            n_t = [None] * NG
            e_t = [None] * NG
            sab_t = [None] * NG
            p_t = [None] * IN_BLOCKS

            def chain(g):
                js, _ = GSPEC[g]
                fd = len(js) * 2 * IK
                w = wt_sb[:, js[0] : js[0] + len(js)].rearrange(flat)
                gg = vpool.tile([P, fd], F16, name="gg16", tag="tv16")
                nc.vector.tensor_scalar(gg[:], w, A2M, P1, MUL, ADD)
                hh = vpool.tile([P, fd], F16, name="hh16", tag="tv16")
                nc.vector.tensor_tensor(hh[:], w, gg[:], MUL)
                n_t[g] = npool.tile([P, fd], U8, name="nidx", tag="nidx")
                nc.gpsimd.tensor_scalar(n_t[g][:], hh[:], P0, None, ADD)

            def exp_stage(g):
                js, _ = GSPEC[g]
                nj = len(js)
                e_t[g] = epool.tile([P, nj, 2, IK], F16, name="texp", tag="texp")
                nc.scalar.activation(
                    e_t[g][:].rearrange(flat),
                    n_t[g][:],
                    mybir.ActivationFunctionType.Exp,
                    bias=0.0,
                    scale=LN_R,
                )

            def mm_stage(g):
                js, _ = GSPEC[g]
                for jrel, j in enumerate(js):
                    p_t[j] = yps.tile([P, IK], F32, name="yp", tag="yp")
                    for h in range(2):
                        nc.tensor.matmul(
                            s2_ps[:, j : j + 1],
                            xt_sb[:, j, h, :],
                            negones_sb[:],
                            start=(h == 0),
                            stop=(h == 1),
                        )
                        nc.tensor.matmul(
                            p_t[j][:],
                            xt_sb[:, j, h, :],
                            e_t[g][:, jrel, h, :],
                            start=(h == 0),
                            stop=(h == 1),
                        )

            def evict_stage(g):
                js, eng = GSPEC[g]
                nj = len(js)
                # s2_ps holds s2x = -s2 (ones preset to -1.0, exact in f16)
                sab_t[g] = spool.tile([P, nj], F32, name="sab", tag="sab")
                if eng == "v":
                    # sa = -A/B*s2, then y = (p + sa)*(-B)
                    nc.vector.tensor_scalar(
                        sab_t[g][:],
                        s2_ps[:, js[0] : js[0] + nj],
                        A_OFF / B_SCALE,
                        None,
                        MUL,
                    )
                    for jrel, j in enumerate(js):
                        nc.vector.tensor_scalar(
                            y_sb[:, j, :],
                            p_t[j][:],
                            sab_t[g][:, jrel : jrel + 1],
                            -B_SCALE,
                            ADD,
                            MUL,
                        )
                else:
                    # sab = A*s2 (SBUF), then ACT: y = -B*p + sab
                    nc.vector.tensor_scalar(
                        sab_t[g][:],
                        s2_ps[:, js[0] : js[0] + nj],
                        -A_OFF,
                        None,
                        MUL,
                    )
                    for jrel, j in enumerate(js):
                        nc.scalar.activation(
                            y_sb[:, j, :],
                            p_t[j][:],
                            mybir.ActivationFunctionType.Identity,
                            bias=sab_t[g][:, jrel : jrel + 1],
                            scale=-B_SCALE,
                        )

            # --- prologue ---
            # tiny Exp forces the ACT table load before real work
            warm = singles.tile([P, 1], F32)
            nc.scalar.activation(
                warm[:], warm[:], mybir.ActivationFunctionType.Exp,
                bias=0.0, scale=0.0,
            )
            negones_sb = singles.tile([P, 1], F16)
            nc.vector.memset(negones_sb[:], -1.0)
            s2_ps = sps.tile([P, IN_BLOCKS], F32)
            warm_l = singles.tile([P, 16], F16)
            nc.vector.memset(warm_l[:], 0.5)
            warm_r = singles.tile([P, IK], F16)
            nc.vector.memset(warm_r[:], 0.5)
            wm_ps = sps.tile([16, IK], F32)

            # single resident tiles for w, x, y; DMAs sliced large
            wt_sb = singles.tile([P, IN_BLOCKS, 2, IK], F16)
            xt_sb = singles.tile([P, IN_BLOCKS, 2, BATCH], F16)
            y_sb = singles.tile([P, IN_BLOCKS, IK], F16)

            for item in DMA_PLAN:
                if item == "x0":
                    nc.gpsimd.dma_start(out=xt_sb[:, 0:8], in_=xt_d[:, 0:8])
                elif item == "x1":
                    nc.gpsimd.dma_start(out=xt_sb[:, 8:16], in_=xt_d[:, 8:16])
                else:
                    j0, nj, ring = item
                    eng = nc.sync if ring == "s" else nc.gpsimd
                    eng.dma_start(
                        out=wt_sb[:, j0 : j0 + nj], in_=wt_d[:, j0 : j0 + nj]
                    )

            # PE warm-up raises the HAM clock gate to 2.4 GHz
            for _ in range(8):
                nc.tensor.matmul(
                    wm_ps[:], warm_l[:], warm_r[:], start=True, stop=True
                )

            # --- pipelined main loop ---
            stores = list(STORES)

            def flush_stores(done_j):
                while stores and stores[0][0] + stores[0][1] <= done_j:
                    j0, nj = stores.pop(0)
                    nc.sync.dma_start(
                        out=y_d[:, j0 : j0 + nj], in_=y_sb[:, j0 : j0 + nj]
                    )

            chain(0)
            chain(1)
            for g in range(NG):
                if g + 2 < NG:
                    chain(g + 2)
                exp_stage(g)
                mm_stage(g)
                if g >= 1:
                    evict_stage(g - 1)
                    flush_stores(GSPEC[g - 1][0][-1] + 1)
            evict_stage(NG - 1)
            flush_stores(IN_BLOCKS)

    with _ForceExpIdentityTable():
        nc.compile()
    return nc


def _get_nc():
    if "nc" not in _CACHE:
        _CACHE["nc"] = _build()
    return _CACHE["nc"]


def kernel(x: np.ndarray, cores: np.ndarray, _trace=False, _trace_kwargs=None):
    x = np.asarray(x, dtype=np.float32)
    cores = np.asarray(cores, dtype=np.float32)

    xt = np.ascontiguousarray(
        x.T.reshape(IN_BLOCKS, 2, 128, BATCH)
        .transpose(2, 0, 1, 3)
        .astype(np.float16)
    )
    wt_full = (
        cores.reshape(OUT_BLOCKS, IN_BLOCKS, NB, 2, 128)  # i, j, k, h, p
        .transpose(4, 1, 3, 0, 2)  # p, j, h, i, k
        .astype(np.float16)
    )

    in_maps = []
    for c in range(N_CORES):
        wc = np.ascontiguousarray(
            wt_full[:, :, :, c * I_PER_CORE : (c + 1) * I_PER_CORE, :]
        ).reshape(128, IN_BLOCKS, 2, IK)
        in_maps.append({"xt": xt, "wt": wc})

    nc = _get_nc()
    kw = {}
    if _trace:
        kw = dict(trace=True, **(_trace_kwargs or {}))
    out = run_bass_kernel_spmd(nc, in_maps, core_ids=list(range(N_CORES)), **kw)
    if _trace:
        _CACHE["last_result"] = out
    y = np.concatenate(
        [
            r["y"]  # (b, j, (i,k))
            .astype(np.float32)
            .reshape(BATCH, IN_BLOCKS, I_PER_CORE, NB)
            .transpose(0, 2, 1, 3)
            for r in out.results
        ],
        axis=1,
    )
    return y
